# revision 8
# baseline (speedup 1.0000x reference)
"""Trainium2 Bass kernel: LocalCausalTransformerBlock (window-3 causal attention).

Sharding: 8-way sequence-parallel. B=2 x N=2048 = 4096 tokens -> 8 chunks of
512 tokens (4 chunks per batch row). Each core gets its 512 tokens plus a
2-token halo (the preceding tokens of the same sequence) so the window-3
causal attention needs no cross-core communication. Weights are replicated.

Device layout: activations live "transposed" (channels on partitions, tokens
on the free axis) so every matmul contracts over partitions and the +-1/+-2
token shifts of the local attention are plain free-axis offsets.

Host-side folds: LayerNorm gamma/beta are folded into the following matmul
weights/bias; the attention scale (1/sqrt(64)) is folded into the Q columns
of qkv_w/qkv_b. Matmul inputs are cast to bf16 (fp32 accumulate in PSUM);
LayerNorm stats, softmax and both residual streams stay fp32.
"""

import sys

for _p in ("/opt/trn_rl_repo",):
    if _p not in sys.path:
        sys.path.insert(0, _p)

import numpy as np
import ml_dtypes

P = 128
D = 1024
H = 16
HD = 64
H3 = 3 * D
HID = 4096
T = 512            # real tokens per core
TH = T + 2         # with 2-token halo (halo stored first)
NCORE = 8
EPS = 1e-5
NEG = -1e30
BF = ml_dtypes.bfloat16

_CACHE: dict = {}


def _build_program():
    import concourse.bass as bass
    import concourse.tile as tile
    from concourse import bacc, mybir
    from contextlib import ExitStack

    f32 = mybir.dt.float32
    bf16 = mybir.dt.bfloat16
    ALU = mybir.AluOpType
    ACT = mybir.ActivationFunctionType

    nc = bacc.Bacc()

    xh_d = nc.declare_dram_parameter("xh", [2, D], f32, isOutput=False)
    xm_d = nc.declare_dram_parameter("xm", [T, D], f32, isOutput=False)
    qkvw_d = nc.declare_dram_parameter("qkvw", [D, H3], bf16, isOutput=False)
    projw_d = nc.declare_dram_parameter("projw", [D, D], bf16, isOutput=False)
    fc1w_d = nc.declare_dram_parameter("fc1w", [D, HID], bf16, isOutput=False)
    fc2w_d = nc.declare_dram_parameter("fc2w", [HID, D], bf16, isOutput=False)
    qkvb_d = nc.declare_dram_parameter("qkvb", [P, 24], f32, isOutput=False)
    projb_d = nc.declare_dram_parameter("projb", [P, 8], f32, isOutput=False)
    fc1b_d = nc.declare_dram_parameter("fc1b", [P, 32], f32, isOutput=False)
    fc2b_d = nc.declare_dram_parameter("fc2b", [P, 8], f32, isOutput=False)
    idf_d = nc.declare_dram_parameter("idf", [P, P], f32, isOutput=False)
    idb_d = nc.declare_dram_parameter("idb", [P, P], bf16, isOutput=False)
    hmask_d = nc.declare_dram_parameter("hmask", [P, 8 * H], bf16, isOutput=False)
    emask_d = nc.declare_dram_parameter("emask", [H, 8 * P], bf16, isOutput=False)
    smask_d = nc.declare_dram_parameter("smask", [H, 3 * T], f32, isOutput=False)
    out_d = nc.declare_dram_parameter("out", [T, D], f32, isOutput=True)

    with tile.TileContext(nc) as tc, ExitStack() as ctx:
        # ---- program-lifetime pools ----
        const = ctx.enter_context(tc.tile_pool(name="const", bufs=1))
        acts = ctx.enter_context(tc.tile_pool(name="acts", bufs=1))
        ln_pool = ctx.enter_context(tc.tile_pool(name="ln", bufs=3))
        tp_ps = ctx.enter_context(tc.tile_pool(name="tp_ps", bufs=3, space="PSUM"))
        mm_ps = ctx.enter_context(tc.tile_pool(name="mm_ps", bufs=5, space="PSUM"))

        idf = const.tile([P, P], f32, tag="c", name="idf")
        nc.sync.dma_start(idf[:], idf_d[:])
        idb = const.tile([P, P], bf16, tag="c2", name="idb")
        nc.sync.dma_start(idb[:], idb_d[:])
        hmask = const.tile([P, 8 * H], bf16, tag="c3", name="hmask")
        nc.sync.dma_start(hmask[:], hmask_d[:])
        emask = const.tile([H, 8 * P], bf16, tag="c4", name="emask")
        nc.sync.dma_start(emask[:], emask_d[:])
        smask = const.tile([H, 3 * T], f32, tag="c5", name="smask")
        nc.sync.dma_start(smask[:], smask_d[:])
        qkvb = const.tile([P, 24], f32, tag="c6", name="qkvb")
        nc.sync.dma_start(qkvb[:], qkvb_d[:])
        projb = const.tile([P, 8], f32, tag="c7", name="projb")
        nc.sync.dma_start(projb[:], projb_d[:])
        fc1b = const.tile([P, 32], f32, tag="c8", name="fc1b")
        nc.sync.dma_start(fc1b[:], fc1b_d[:])
        fc2b = const.tile([P, 8], f32, tag="c9", name="fc2b")
        nc.sync.dma_start(fc2b[:], fc2b_d[:])

        # activations alive into the MLP phases
        x2t = acts.tile([P, 4 * D], f32, tag="x2t", name="x2t")
        x2lnT = acts.tile([P, 8 * T], bf16, tag="x2lnT", name="x2lnT")
        hT = acts.tile([P, 32 * T], bf16, tag="hT", name="hT")

        def layernorm_T(src_ap, s, dstT, dst_col, dst_stride):
            stat = ln_pool.tile([s, 12], f32, tag=f"lnstat{s}", name=f"st{s}")
            nc.vector.bn_stats(stat[:, 0:6], src_ap[:, 0:512])
            nc.vector.bn_stats(stat[:, 6:12], src_ap[:, 512:1024])
            mv = ln_pool.tile([s, 2], f32, tag=f"lnmv{s}", name=f"mv{s}")
            nc.vector.bn_aggr(mv[:], stat[:])
            vpe = ln_pool.tile([s, 1], f32, tag=f"lnvpe{s}", name=f"vpe{s}")
            nc.vector.tensor_scalar_add(vpe[:], mv[:, 1:2], EPS)
            std = ln_pool.tile([s, 1], f32, tag=f"lnstd{s}", name=f"sd{s}")
            nc.scalar.activation(std[:], vpe[:], ACT.Sqrt)
            rstd = ln_pool.tile([s, 1], f32, tag=f"lnrstd{s}", name=f"rs{s}")
            nc.vector.reciprocal(rstd[:], std[:])
            nmr = ln_pool.tile([s, 1], f32, tag=f"lnnmr{s}", name=f"nm{s}")
            nc.vector.scalar_tensor_tensor(
                nmr[:], mv[:, 0:1], -1.0, rstd[:], ALU.mult, ALU.mult
            )
            xln = ln_pool.tile([s, D], bf16, tag=f"lnout{s}", name=f"xo{s}")
            nc.scalar.activation(
                xln[:], src_ap[:], ACT.Identity, bias=nmr[:, 0:1], scale=rstd[:, 0:1]
            )
            for ch in range(8):
                tp = tp_ps.tile([P, s], bf16, tag="tp", name=f"tpl{s}_{ch}")
                nc.tensor.transpose(tp[:], xln[:, ch * P:(ch + 1) * P], idb[0:s, 0:s])
                c0 = ch * dst_stride + dst_col
                nc.vector.tensor_copy(dstT[:, c0:c0 + s], tp[:])

        with tc.tile_pool(name="p1", bufs=1) as p1:
            xt = p1.tile([P, 4 * D], f32, tag="xt", name="xt")
            xh = p1.tile([2, D], f32, tag="xh", name="xh")
            xlnT = p1.tile([P, 8 * TH], bf16, tag="xlnT", name="xlnT")
            qT = p1.tile([P, 8 * T], bf16, tag="qT", name="qT")
            kT = p1.tile([P, 8 * TH], bf16, tag="kT", name="kT")
            vT = p1.tile([P, 8 * TH], f32, tag="vT", name="vT")

            for ti in range(4):
                nc.sync.dma_start(xt[:, ti * D:(ti + 1) * D],
                                  xm_d[ti * P:(ti + 1) * P, :])
            nc.sync.dma_start(xh[:], xh_d[:])

            # ---- LN1 (halo + 4 token tiles) ----
            layernorm_T(xh[:], 2, xlnT, 0, TH)
            for ti in range(4):
                layernorm_T(xt[:, ti * D:(ti + 1) * D], P, xlnT, 2 + ti * P, TH)

            # ---- QKV ----
            with tc.tile_pool(name="wq", bufs=1) as wq_pool:
                qslab = []
                for c in range(8):
                    s = wq_pool.tile([P, H3], bf16, tag=f"qw{c}", name=f"qw{c}")
                    nc.sync.dma_start(s[:], qkvw_d[c * P:(c + 1) * P, :])
                    qslab.append(s)

                for j in range(24):
                    ps = mm_ps.tile([P, T], f32, tag="mm", name=f"qkv{j}")
                    for c in range(8):
                        nc.tensor.matmul(
                            ps[:], qslab[c][:, j * P:(j + 1) * P],
                            xlnT[:, c * TH + 2:c * TH + TH],
                            start=(c == 0), stop=(c == 7),
                        )
                    bias = qkvb[:, j:j + 1]
                    if j < 8:
                        dst = qT[:, j * T:(j + 1) * T]
                    elif j < 16:
                        dst = kT[:, (j - 8) * TH + 2:(j - 8) * TH + TH]
                    else:
                        dst = vT[:, (j - 16) * TH + 2:(j - 16) * TH + TH]
                    nc.scalar.activation(dst, ps[:], ACT.Identity, bias=bias)
                    if j >= 8:  # halo K/V columns
                        ph = tp_ps.tile([P, 2], f32, tag="tp", name=f"halo{j}")
                        for c in range(8):
                            nc.tensor.matmul(
                                ph[:], qslab[c][:, j * P:(j + 1) * P],
                                xlnT[:, c * TH:c * TH + 2],
                                start=(c == 0), stop=(c == 7),
                            )
                        if j < 16:
                            hdst = kT[:, (j - 8) * TH:(j - 8) * TH + 2]
                        else:
                            hdst = vT[:, (j - 16) * TH:(j - 16) * TH + 2]
                        nc.scalar.activation(hdst, ph[:], ACT.Identity, bias=bias)

            # ---- attention ----
            with tc.tile_pool(name="p3", bufs=1) as p3:
                attnT = p3.tile([P, 8 * T], bf16, tag="attnT", name="attnT")
                with tc.tile_pool(name="p3b", bufs=1) as p3b:
                    s_sb = p3b.tile([H, 3 * T], f32, tag="s_sb", name="s_sb")
                    for w in range(3):
                        sc = mm_ps.tile([H, T], f32, tag="mm", name=f"sc{w}")
                        for ch in range(8):
                            e = p3b.tile([P, T], bf16, tag="e", bufs=3, name=f"e{w}_{ch}")
                            nc.vector.tensor_mul(
                                e[:], qT[:, ch * T:(ch + 1) * T],
                                kT[:, ch * TH + 2 - w:ch * TH + TH - w],
                            )
                            nc.tensor.matmul(
                                sc[:], hmask[:, ch * H:(ch + 1) * H], e[:],
                                start=(ch == 0), stop=(ch == 7),
                            )
                        nc.vector.tensor_add(
                            s_sb[:, w * T:(w + 1) * T], sc[:],
                            smask[:, w * T:(w + 1) * T]
                        )
                    mx = p3b.tile([H, T], f32, tag="mx", name="mx")
                    mx2 = p3b.tile([H, T], f32, tag="mx2", name="mx2")
                    nc.vector.tensor_max(mx[:], s_sb[:, 0:T], s_sb[:, T:2 * T])
                    nc.vector.tensor_max(mx2[:], mx[:], s_sb[:, 2 * T:3 * T])
                    st2 = p3b.tile([H, 3 * T], f32, tag="st2", name="st2")
                    et = p3b.tile([H, 3 * T], f32, tag="et", name="et")
                    for w in range(3):
                        nc.vector.tensor_sub(st2[:, w * T:(w + 1) * T],
                                             s_sb[:, w * T:(w + 1) * T], mx2[:])
                        nc.scalar.activation(et[:, w * T:(w + 1) * T],
                                             st2[:, w * T:(w + 1) * T], ACT.Exp)
                    z0 = p3b.tile([H, T], f32, tag="z0", name="z0")
                    z1 = p3b.tile([H, T], f32, tag="z1", name="z1")
                    rz = p3b.tile([H, T], f32, tag="rz", name="rz")
                    nc.vector.tensor_add(z0[:], et[:, 0:T], et[:, T:2 * T])
                    nc.vector.tensor_add(z1[:], z0[:], et[:, 2 * T:3 * T])
                    nc.vector.reciprocal(rz[:], z1[:])
                    pw = p3b.tile([H, 3 * T], bf16, tag="pw", name="pw")
                    for w in range(3):
                        nc.vector.tensor_mul(pw[:, w * T:(w + 1) * T],
                                             et[:, w * T:(w + 1) * T], rz[:])

                    for ch in range(8):
                        avs = []
                        for w in range(3):
                            bc = mm_ps.tile([P, T], f32, tag="mm", name=f"bc{ch}_{w}")
                            nc.tensor.matmul(
                                bc[:], emask[:, ch * P:(ch + 1) * P],
                                pw[:, w * T:(w + 1) * T],
                                start=True, stop=True,
                            )
                            av = p3b.tile([P, T], f32, tag="av", bufs=4,
                                          name=f"av{ch}_{w}")
                            nc.vector.tensor_mul(
                                av[:], bc[:], vT[:, ch * TH + 2 - w:ch * TH + TH - w]
                            )
                            avs.append(av)
                        av01 = p3b.tile([P, T], f32, tag="av01", bufs=2,
                                        name=f"av01_{ch}")
                        nc.vector.tensor_add(av01[:], avs[0][:], avs[1][:])
                        nc.vector.tensor_add(attnT[:, ch * T:(ch + 1) * T],
                                             av01[:], avs[2][:])

                # ---- proj + residual 1 + LN2 ----
                with tc.tile_pool(name="p5", bufs=1) as p5:
                    pslab = []
                    for c in range(8):
                        s = p5.tile([P, D], bf16, tag=f"pw{c}", name=f"pjw{c}")
                        nc.sync.dma_start(s[:], projw_d[c * P:(c + 1) * P, :])
                        pslab.append(s)
                    yT = p5.tile([P, 8 * T], f32, tag="yT", name="yT")
                    for j in range(8):
                        ps = mm_ps.tile([P, T], f32, tag="mm", name=f"pj{j}")
                        for c in range(8):
                            nc.tensor.matmul(
                                ps[:], pslab[c][:, j * P:(j + 1) * P],
                                attnT[:, c * T:(c + 1) * T],
                                start=(c == 0), stop=(c == 7),
                            )
                        nc.scalar.activation(yT[:, j * T:(j + 1) * T], ps[:],
                                             ACT.Identity, bias=projb[:, j:j + 1])
                    for ti in range(4):
                        for ch in range(8):
                            tp = tp_ps.tile([P, P], f32, tag="tp", name=f"tpy{ti}_{ch}")
                            nc.tensor.transpose(
                                tp[:], yT[:, ch * T + ti * P:ch * T + (ti + 1) * P],
                                idf[:])
                            nc.vector.tensor_add(
                                x2t[:, ti * D + ch * P:ti * D + (ch + 1) * P],
                                xt[:, ti * D + ch * P:ti * D + (ch + 1) * P], tp[:],
                            )
                        layernorm_T(x2t[:, ti * D:(ti + 1) * D], P, x2lnT, ti * P, T)

        # ---- MLP fc1 + gelu ----
        with tc.tile_pool(name="w1", bufs=1) as w1_pool:
            f1slab = []
            for c in range(8):
                s = w1_pool.tile([P, HID], bf16, tag=f"f1w{c}", name=f"f1w{c}")
                nc.sync.dma_start(s[:], fc1w_d[c * P:(c + 1) * P, :])
                f1slab.append(s)
            for j in range(32):
                ps = mm_ps.tile([P, T], f32, tag="mm", name=f"f1{j}")
                for c in range(8):
                    nc.tensor.matmul(
                        ps[:], f1slab[c][:, j * P:(j + 1) * P],
                        x2lnT[:, c * T:(c + 1) * T],
                        start=(c == 0), stop=(c == 7),
                    )
                nc.scalar.activation(hT[:, j * T:(j + 1) * T], ps[:], ACT.Gelu,
                                     bias=fc1b[:, j:j + 1])

        # ---- fc2 + residual 2 + store ----
        with tc.tile_pool(name="w2", bufs=1) as w2_pool:
            outt = w2_pool.tile([P, 4 * D], f32, tag="outt", name="outt")
            mlp_written = set()
            for jg in range(2):
                pss = [mm_ps.tile([P, T], f32, tag="mm", name=f"mm4_{jg}_{j}")
                       for j in range(4)]
                for c in range(32):
                    slab = w2_pool.tile([P, D], bf16, tag="f2w", bufs=6,
                                        name=f"f2w{jg}_{c}")
                    nc.sync.dma_start(slab[:], fc2w_d[c * P:(c + 1) * P, :])
                    for j in range(4):
                        nc.tensor.matmul(
                            pss[j][:], slab[:, (jg * 4 + j) * P:(jg * 4 + j + 1) * P],
                            hT[:, c * T:(c + 1) * T],
                            start=(c == 0), stop=(c == 31),
                        )
                for j in range(4):
                    jj = jg * 4 + j
                    mlpt = w2_pool.tile([P, T], f32, tag="mlpt", bufs=2,
                                        name=f"mlpt{jj}")
                    nc.scalar.activation(mlpt[:], pss[j][:], ACT.Identity,
                                         bias=fc2b[:, jj:jj + 1])
                    for ti in range(4):
                        tp = tp_ps.tile([P, P], f32, tag="tp", name=f"tpm{jj}_{ti}")
                        nc.tensor.transpose(tp[:], mlpt[:, ti * P:(ti + 1) * P],
                                            idf[:])
                        nc.vector.tensor_add(
                            outt[:, ti * D + jj * P:ti * D + (jj + 1) * P],
                            x2t[:, ti * D + jj * P:ti * D + (jj + 1) * P], tp[:],
                        )
            for ti in range(4):
                nc.sync.dma_start(out_d[ti * P:(ti + 1) * P, :],
                                  outt[:, ti * D:(ti + 1) * D])

    if not nc.is_finalized():
        nc.finalize()
    return nc


def _host_inputs(x, qkv_w, qkv_b, proj_w, proj_b, g1, b1, g2, b2,
                 fc1_w, fc1_b, fc2_w, fc2_b):
    """Build the 8 per-core input maps (fold LN affine + attn scale)."""
    scale = HD ** -0.5
    qkvw_eff = (qkv_w * g1[:, None]).astype(np.float32).copy()
    qkvb_eff = (qkv_b + b1 @ qkv_w).astype(np.float32).copy()
    qkvw_eff[:, 0:D] *= scale
    qkvb_eff[0:D] *= scale
    fc1w_eff = (fc1_w * g2[:, None]).astype(np.float32)
    fc1b_eff = (fc1_b + b2 @ fc1_w).astype(np.float32)

    common = {
        "qkvw": qkvw_eff.astype(BF),
        "projw": proj_w.astype(BF),
        "fc1w": fc1w_eff.astype(BF),
        "fc2w": fc2_w.astype(BF),
        "qkvb": qkvb_eff.reshape(24, P).T.copy(),
        "projb": proj_b.astype(np.float32).reshape(8, P).T.copy(),
        "fc1b": fc1b_eff.reshape(32, P).T.copy(),
        "fc2b": fc2_b.astype(np.float32).reshape(8, P).T.copy(),
        "idf": np.eye(P, dtype=np.float32),
        "idb": np.eye(P, dtype=np.float32).astype(BF),
    }
    hm = np.zeros((P, 8, H), np.float32)
    for c in range(P):
        for ch in range(8):
            hm[c, ch, 2 * ch + c // HD] = 1.0
    common["hmask"] = hm.reshape(P, 8 * H).astype(BF)
    em = np.zeros((H, 8, P), np.float32)
    for ch in range(8):
        for m in range(P):
            em[2 * ch + m // HD, ch, m] = 1.0
    common["emask"] = em.reshape(H, 8 * P).astype(BF)

    sm0 = np.zeros((H, 3, T), np.float32)
    smq0 = sm0.copy()
    smq0[:, 1, 0] = NEG
    smq0[:, 2, 0:2] = NEG

    in_maps = []
    for core in range(NCORE):
        b, q = divmod(core, 4)
        xm = np.ascontiguousarray(x[b, q * T:(q + 1) * T, :], dtype=np.float32)
        if q == 0:
            xhv = np.zeros((2, D), np.float32)
        else:
            xhv = np.ascontiguousarray(x[b, q * T - 2:q * T, :], dtype=np.float32)
        m = dict(common)
        m["xm"] = xm
        m["xh"] = xhv
        m["smask"] = (smq0 if q == 0 else sm0).reshape(H, 3 * T).copy()
        in_maps.append(m)
    return in_maps


def kernel(**inputs) -> np.ndarray:
    from concourse.bass_utils import run_bass_kernel_spmd

    if "nc" not in _CACHE:
        _CACHE["nc"] = _build_program()
    nc = _CACHE["nc"]
    in_maps = _host_inputs(**inputs)
    res = run_bass_kernel_spmd(nc, in_maps, list(range(NCORE)))
    outs = res.results
    full = np.zeros((2, 2048, D), np.float32)
    for core in range(NCORE):
        b, q = divmod(core, 4)
        full[b, q * T:(q + 1) * T, :] = outs[core]["out"]
    return full


# revision 10
# speedup vs baseline: 340.6569x; 340.6569x over previous
"""Trainium2 Bass kernel: LocalCausalTransformerBlock (window-3 causal attention).

Sharding: 8-way sequence-parallel. B=2 x N=2048 = 4096 tokens -> 8 chunks of
512 tokens (4 chunks per batch row). Each core gets its 512 tokens plus a
2-token halo (the preceding tokens of the same sequence) so the window-3
causal attention needs no cross-core communication. Weights are replicated.

Device layout: activations live "transposed" (channels on partitions, tokens
on the free axis) so every matmul contracts over partitions and the +-1/+-2
token shifts of the local attention are plain free-axis offsets.

Host-side folds: LayerNorm gamma/beta are folded into the following matmul
weights/bias; the attention scale (1/sqrt(64)) is folded into the Q columns
of qkv_w/qkv_b. Matmul inputs are cast to bf16 (fp32 accumulate in PSUM);
LayerNorm stats, softmax and both residual streams stay fp32.
"""

import sys

for _p in ("/opt/trn_rl_repo",):
    if _p not in sys.path:
        sys.path.insert(0, _p)

import numpy as np
import ml_dtypes

P = 128
D = 1024
H = 16
HD = 64
H3 = 3 * D
HID = 4096
T = 512            # real tokens per core
TH = T + 2         # with 2-token halo (halo stored first)
NCORE = 8
EPS = 1e-5
NEG = -1e30
BF = ml_dtypes.bfloat16

_CACHE: dict = {}


def _build_program():
    import concourse.bass as bass
    import concourse.tile as tile
    from concourse import bacc, mybir
    from contextlib import ExitStack

    f32 = mybir.dt.float32
    bf16 = mybir.dt.bfloat16
    ALU = mybir.AluOpType
    ACT = mybir.ActivationFunctionType

    nc = bacc.Bacc()

    xh_d = nc.declare_dram_parameter("xh", [2, D], f32, isOutput=False)
    xm_d = nc.declare_dram_parameter("xm", [T, D], f32, isOutput=False)
    qkvw_d = nc.declare_dram_parameter("qkvw", [D, H3], bf16, isOutput=False)
    projw_d = nc.declare_dram_parameter("projw", [D, D], bf16, isOutput=False)
    fc1w_d = nc.declare_dram_parameter("fc1w", [D, HID], bf16, isOutput=False)
    fc2w_d = nc.declare_dram_parameter("fc2w", [HID, D], bf16, isOutput=False)
    qkvb_d = nc.declare_dram_parameter("qkvb", [P, 24], f32, isOutput=False)
    projb_d = nc.declare_dram_parameter("projb", [P, 8], f32, isOutput=False)
    fc1b_d = nc.declare_dram_parameter("fc1b", [P, 32], f32, isOutput=False)
    fc2b_d = nc.declare_dram_parameter("fc2b", [P, 8], f32, isOutput=False)
    idf_d = nc.declare_dram_parameter("idf", [P, P], f32, isOutput=False)
    idb_d = nc.declare_dram_parameter("idb", [P, P], bf16, isOutput=False)
    hmask_d = nc.declare_dram_parameter("hmask", [P, 8 * H], bf16, isOutput=False)
    emask_d = nc.declare_dram_parameter("emask", [H, 8 * P], bf16, isOutput=False)
    smask_d = nc.declare_dram_parameter("smask", [H, 3 * T], f32, isOutput=False)
    out_d = nc.declare_dram_parameter("out", [T, D], f32, isOutput=True)

    with tile.TileContext(nc) as tc, ExitStack() as ctx:
        # ---- program-lifetime pools ----
        const = ctx.enter_context(tc.tile_pool(name="const", bufs=1))
        acts = ctx.enter_context(tc.tile_pool(name="acts", bufs=1))
        ln_pool = ctx.enter_context(tc.tile_pool(name="ln", bufs=3))
        tp_ps = ctx.enter_context(tc.tile_pool(name="tp_ps", bufs=3, space="PSUM"))
        mm_ps = ctx.enter_context(tc.tile_pool(name="mm_ps", bufs=5, space="PSUM"))

        idf = const.tile([P, P], f32, tag="c", name="idf")
        nc.sync.dma_start(idf[:], idf_d[:])
        idb = const.tile([P, P], bf16, tag="c2", name="idb")
        nc.sync.dma_start(idb[:], idb_d[:])
        hmask = const.tile([P, 8 * H], bf16, tag="c3", name="hmask")
        nc.sync.dma_start(hmask[:], hmask_d[:])
        emask = const.tile([H, 8 * P], bf16, tag="c4", name="emask")
        nc.sync.dma_start(emask[:], emask_d[:])
        smask = const.tile([H, 3 * T], f32, tag="c5", name="smask")
        nc.sync.dma_start(smask[:], smask_d[:])
        qkvb = const.tile([P, 24], f32, tag="c6", name="qkvb")
        nc.sync.dma_start(qkvb[:], qkvb_d[:])
        projb = const.tile([P, 8], f32, tag="c7", name="projb")
        nc.sync.dma_start(projb[:], projb_d[:])
        fc1b = const.tile([P, 32], f32, tag="c8", name="fc1b")
        nc.sync.dma_start(fc1b[:], fc1b_d[:])
        fc2b = const.tile([P, 8], f32, tag="c9", name="fc2b")
        nc.sync.dma_start(fc2b[:], fc2b_d[:])

        # activations alive into the MLP phases
        x2t = acts.tile([P, 4 * D], f32, tag="x2t", name="x2t")
        x2lnT = acts.tile([P, 8 * T], bf16, tag="x2lnT", name="x2lnT")
        hT = acts.tile([P, 32 * T], bf16, tag="hT", name="hT")

        def layernorm_T(src_ap, s, dstT, dst_col, dst_stride):
            stat = ln_pool.tile([s, 12], f32, tag=f"lnstat{s}", name=f"st{s}")
            nc.vector.bn_stats(stat[:, 0:6], src_ap[:, 0:512])
            nc.vector.bn_stats(stat[:, 6:12], src_ap[:, 512:1024])
            mv = ln_pool.tile([s, 2], f32, tag=f"lnmv{s}", name=f"mv{s}")
            nc.vector.bn_aggr(mv[:], stat[:])
            vpe = ln_pool.tile([s, 1], f32, tag=f"lnvpe{s}", name=f"vpe{s}")
            nc.vector.tensor_scalar_add(vpe[:], mv[:, 1:2], EPS)
            std = ln_pool.tile([s, 1], f32, tag=f"lnstd{s}", name=f"sd{s}")
            nc.scalar.activation(std[:], vpe[:], ACT.Sqrt)
            rstd = ln_pool.tile([s, 1], f32, tag=f"lnrstd{s}", name=f"rs{s}")
            nc.vector.reciprocal(rstd[:], std[:])
            nmr = ln_pool.tile([s, 1], f32, tag=f"lnnmr{s}", name=f"nm{s}")
            nc.vector.scalar_tensor_tensor(
                nmr[:], mv[:, 0:1], -1.0, rstd[:], ALU.mult, ALU.mult
            )
            xln = ln_pool.tile([s, D], bf16, tag=f"lnout{s}", name=f"xo{s}")
            nc.scalar.activation(
                xln[:], src_ap[:], ACT.Identity, bias=nmr[:, 0:1], scale=rstd[:, 0:1]
            )
            for ch in range(8):
                tp = tp_ps.tile([P, s], bf16, tag="tp", name=f"tpl{s}_{ch}")
                nc.tensor.transpose(tp[:], xln[:, ch * P:(ch + 1) * P], idb[0:s, 0:s])
                c0 = ch * dst_stride + dst_col
                nc.vector.tensor_copy(dstT[:, c0:c0 + s], tp[:])

        with tc.tile_pool(name="p1", bufs=1) as p1:
            xt = p1.tile([P, 4 * D], f32, tag="xt", name="xt")
            xh = p1.tile([2, D], f32, tag="xh", name="xh")
            xlnT = p1.tile([P, 8 * TH], bf16, tag="xlnT", name="xlnT")
            qT = p1.tile([P, 8 * T], bf16, tag="qT", name="qT")
            kT = p1.tile([P, 8 * TH], bf16, tag="kT", name="kT")
            vT = p1.tile([P, 8 * TH], f32, tag="vT", name="vT")

            for ti in range(4):
                nc.sync.dma_start(xt[:, ti * D:(ti + 1) * D],
                                  xm_d[ti * P:(ti + 1) * P, :])
            nc.sync.dma_start(xh[:], xh_d[:])

            # ---- LN1 (halo + 4 token tiles) ----
            layernorm_T(xh[:], 2, xlnT, 0, TH)
            for ti in range(4):
                layernorm_T(xt[:, ti * D:(ti + 1) * D], P, xlnT, 2 + ti * P, TH)

            # ---- QKV ----
            with tc.tile_pool(name="wq", bufs=1) as wq_pool:
                qslab = []
                for c in range(8):
                    s = wq_pool.tile([P, H3], bf16, tag=f"qw{c}", name=f"qw{c}")
                    nc.sync.dma_start(s[:], qkvw_d[c * P:(c + 1) * P, :])
                    qslab.append(s)

                for j in range(24):
                    ps = mm_ps.tile([P, T], f32, tag="mm", name=f"qkv{j}")
                    for c in range(8):
                        nc.tensor.matmul(
                            ps[:], qslab[c][:, j * P:(j + 1) * P],
                            xlnT[:, c * TH + 2:c * TH + TH],
                            start=(c == 0), stop=(c == 7),
                        )
                    bias = qkvb[:, j:j + 1]
                    if j < 8:
                        dst = qT[:, j * T:(j + 1) * T]
                    elif j < 16:
                        dst = kT[:, (j - 8) * TH + 2:(j - 8) * TH + TH]
                    else:
                        dst = vT[:, (j - 16) * TH + 2:(j - 16) * TH + TH]
                    nc.scalar.activation(dst, ps[:], ACT.Identity, bias=bias)
                    if j >= 8:  # halo K/V columns
                        ph = tp_ps.tile([P, 2], f32, tag="tp", name=f"halo{j}")
                        for c in range(8):
                            nc.tensor.matmul(
                                ph[:], qslab[c][:, j * P:(j + 1) * P],
                                xlnT[:, c * TH:c * TH + 2],
                                start=(c == 0), stop=(c == 7),
                            )
                        if j < 16:
                            hdst = kT[:, (j - 8) * TH:(j - 8) * TH + 2]
                        else:
                            hdst = vT[:, (j - 16) * TH:(j - 16) * TH + 2]
                        nc.scalar.activation(hdst, ph[:], ACT.Identity, bias=bias)

            # ---- attention ----
            with tc.tile_pool(name="p3", bufs=1) as p3:
                attnT = p3.tile([P, 8 * T], bf16, tag="attnT", name="attnT")
                with tc.tile_pool(name="p3b", bufs=1) as p3b:
                    s_sb = p3b.tile([H, 3 * T], f32, tag="s_sb", name="s_sb")
                    for w in range(3):
                        sc = mm_ps.tile([H, T], f32, tag="mm", name=f"sc{w}")
                        for ch in range(8):
                            e = p3b.tile([P, T], bf16, tag="e", bufs=3, name=f"e{w}_{ch}")
                            nc.vector.tensor_mul(
                                e[:], qT[:, ch * T:(ch + 1) * T],
                                kT[:, ch * TH + 2 - w:ch * TH + TH - w],
                            )
                            nc.tensor.matmul(
                                sc[:], hmask[:, ch * H:(ch + 1) * H], e[:],
                                start=(ch == 0), stop=(ch == 7),
                            )
                        nc.vector.tensor_add(
                            s_sb[:, w * T:(w + 1) * T], sc[:],
                            smask[:, w * T:(w + 1) * T]
                        )
                    mx = p3b.tile([H, T], f32, tag="mx", name="mx")
                    mx2 = p3b.tile([H, T], f32, tag="mx2", name="mx2")
                    nc.vector.tensor_max(mx[:], s_sb[:, 0:T], s_sb[:, T:2 * T])
                    nc.vector.tensor_max(mx2[:], mx[:], s_sb[:, 2 * T:3 * T])
                    st2 = p3b.tile([H, 3 * T], f32, tag="st2", name="st2")
                    et = p3b.tile([H, 3 * T], f32, tag="et", name="et")
                    for w in range(3):
                        nc.vector.tensor_sub(st2[:, w * T:(w + 1) * T],
                                             s_sb[:, w * T:(w + 1) * T], mx2[:])
                        nc.scalar.activation(et[:, w * T:(w + 1) * T],
                                             st2[:, w * T:(w + 1) * T], ACT.Exp)
                    z0 = p3b.tile([H, T], f32, tag="z0", name="z0")
                    z1 = p3b.tile([H, T], f32, tag="z1", name="z1")
                    rz = p3b.tile([H, T], f32, tag="rz", name="rz")
                    nc.vector.tensor_add(z0[:], et[:, 0:T], et[:, T:2 * T])
                    nc.vector.tensor_add(z1[:], z0[:], et[:, 2 * T:3 * T])
                    nc.vector.reciprocal(rz[:], z1[:])
                    pw = p3b.tile([H, 3 * T], bf16, tag="pw", name="pw")
                    for w in range(3):
                        nc.vector.tensor_mul(pw[:, w * T:(w + 1) * T],
                                             et[:, w * T:(w + 1) * T], rz[:])

                    for ch in range(8):
                        avs = []
                        for w in range(3):
                            bc = mm_ps.tile([P, T], f32, tag="mm", name=f"bc{ch}_{w}")
                            nc.tensor.matmul(
                                bc[:], emask[:, ch * P:(ch + 1) * P],
                                pw[:, w * T:(w + 1) * T],
                                start=True, stop=True,
                            )
                            av = p3b.tile([P, T], f32, tag="av", bufs=4,
                                          name=f"av{ch}_{w}")
                            nc.vector.tensor_mul(
                                av[:], bc[:], vT[:, ch * TH + 2 - w:ch * TH + TH - w]
                            )
                            avs.append(av)
                        av01 = p3b.tile([P, T], f32, tag="av01", bufs=2,
                                        name=f"av01_{ch}")
                        nc.vector.tensor_add(av01[:], avs[0][:], avs[1][:])
                        nc.vector.tensor_add(attnT[:, ch * T:(ch + 1) * T],
                                             av01[:], avs[2][:])

                # ---- proj + residual 1 + LN2 ----
                with tc.tile_pool(name="p5", bufs=1) as p5:
                    pslab = []
                    for c in range(8):
                        s = p5.tile([P, D], bf16, tag=f"pw{c}", name=f"pjw{c}")
                        nc.sync.dma_start(s[:], projw_d[c * P:(c + 1) * P, :])
                        pslab.append(s)
                    yT = p5.tile([P, 8 * T], f32, tag="yT", name="yT")
                    for j in range(8):
                        ps = mm_ps.tile([P, T], f32, tag="mm", name=f"pj{j}")
                        for c in range(8):
                            nc.tensor.matmul(
                                ps[:], pslab[c][:, j * P:(j + 1) * P],
                                attnT[:, c * T:(c + 1) * T],
                                start=(c == 0), stop=(c == 7),
                            )
                        nc.scalar.activation(yT[:, j * T:(j + 1) * T], ps[:],
                                             ACT.Identity, bias=projb[:, j:j + 1])
                    for ti in range(4):
                        for ch in range(8):
                            tp = tp_ps.tile([P, P], f32, tag="tp", name=f"tpy{ti}_{ch}")
                            nc.tensor.transpose(
                                tp[:], yT[:, ch * T + ti * P:ch * T + (ti + 1) * P],
                                idf[:])
                            nc.vector.tensor_add(
                                x2t[:, ti * D + ch * P:ti * D + (ch + 1) * P],
                                xt[:, ti * D + ch * P:ti * D + (ch + 1) * P], tp[:],
                            )
                        layernorm_T(x2t[:, ti * D:(ti + 1) * D], P, x2lnT, ti * P, T)

        # ---- MLP fc1 + gelu ----
        with tc.tile_pool(name="w1", bufs=1) as w1_pool:
            f1slab = []
            for c in range(8):
                s = w1_pool.tile([P, HID], bf16, tag=f"f1w{c}", name=f"f1w{c}")
                nc.sync.dma_start(s[:], fc1w_d[c * P:(c + 1) * P, :])
                f1slab.append(s)
            for j in range(32):
                ps = mm_ps.tile([P, T], f32, tag="mm", name=f"f1{j}")
                for c in range(8):
                    nc.tensor.matmul(
                        ps[:], f1slab[c][:, j * P:(j + 1) * P],
                        x2lnT[:, c * T:(c + 1) * T],
                        start=(c == 0), stop=(c == 7),
                    )
                nc.scalar.activation(hT[:, j * T:(j + 1) * T], ps[:], ACT.Gelu,
                                     bias=fc1b[:, j:j + 1])

        # ---- fc2 + residual 2 + store ----
        with tc.tile_pool(name="w2", bufs=1) as w2_pool:
            outt = w2_pool.tile([P, 4 * D], f32, tag="outt", name="outt")
            mlp_written = set()
            for jg in range(2):
                pss = [mm_ps.tile([P, T], f32, tag="mm", name=f"mm4_{jg}_{j}")
                       for j in range(4)]
                for c in range(32):
                    slab = w2_pool.tile([P, D], bf16, tag="f2w", bufs=6,
                                        name=f"f2w{jg}_{c}")
                    nc.sync.dma_start(slab[:], fc2w_d[c * P:(c + 1) * P, :])
                    for j in range(4):
                        nc.tensor.matmul(
                            pss[j][:], slab[:, (jg * 4 + j) * P:(jg * 4 + j + 1) * P],
                            hT[:, c * T:(c + 1) * T],
                            start=(c == 0), stop=(c == 31),
                        )
                for j in range(4):
                    jj = jg * 4 + j
                    mlpt = w2_pool.tile([P, T], f32, tag="mlpt", bufs=2,
                                        name=f"mlpt{jj}")
                    nc.scalar.activation(mlpt[:], pss[j][:], ACT.Identity,
                                         bias=fc2b[:, jj:jj + 1])
                    for ti in range(4):
                        tp = tp_ps.tile([P, P], f32, tag="tp", name=f"tpm{jj}_{ti}")
                        nc.tensor.transpose(tp[:], mlpt[:, ti * P:(ti + 1) * P],
                                            idf[:])
                        nc.vector.tensor_add(
                            outt[:, ti * D + jj * P:ti * D + (jj + 1) * P],
                            x2t[:, ti * D + jj * P:ti * D + (jj + 1) * P], tp[:],
                        )
            for ti in range(4):
                nc.sync.dma_start(out_d[ti * P:(ti + 1) * P, :],
                                  outt[:, ti * D:(ti + 1) * D])

    if not nc.is_finalized():
        nc.finalize()
    return nc


def _host_inputs(x, qkv_w, qkv_b, proj_w, proj_b, g1, b1, g2, b2,
                 fc1_w, fc1_b, fc2_w, fc2_b):
    """Build the 8 per-core input maps (fold LN affine + attn scale)."""
    scale = HD ** -0.5
    qkvw_eff = (qkv_w * g1[:, None]).astype(np.float32).copy()
    qkvb_eff = (qkv_b + b1 @ qkv_w).astype(np.float32).copy()
    qkvw_eff[:, 0:D] *= scale
    qkvb_eff[0:D] *= scale
    fc1w_eff = (fc1_w * g2[:, None]).astype(np.float32)
    fc1b_eff = (fc1_b + b2 @ fc1_w).astype(np.float32)

    common = {
        "qkvw": qkvw_eff.astype(BF),
        "projw": proj_w.astype(BF),
        "fc1w": fc1w_eff.astype(BF),
        "fc2w": fc2_w.astype(BF),
        "qkvb": qkvb_eff.reshape(24, P).T.copy(),
        "projb": proj_b.astype(np.float32).reshape(8, P).T.copy(),
        "fc1b": fc1b_eff.reshape(32, P).T.copy(),
        "fc2b": fc2_b.astype(np.float32).reshape(8, P).T.copy(),
        "idf": np.eye(P, dtype=np.float32),
        "idb": np.eye(P, dtype=np.float32).astype(BF),
    }
    hm = np.zeros((P, 8, H), np.float32)
    for c in range(P):
        for ch in range(8):
            hm[c, ch, 2 * ch + c // HD] = 1.0
    common["hmask"] = hm.reshape(P, 8 * H).astype(BF)
    em = np.zeros((H, 8, P), np.float32)
    for ch in range(8):
        for m in range(P):
            em[2 * ch + m // HD, ch, m] = 1.0
    common["emask"] = em.reshape(H, 8 * P).astype(BF)

    sm0 = np.zeros((H, 3, T), np.float32)
    smq0 = sm0.copy()
    smq0[:, 1, 0] = NEG
    smq0[:, 2, 0:2] = NEG

    in_maps = []
    for core in range(NCORE):
        b, q = divmod(core, 4)
        xm = np.ascontiguousarray(x[b, q * T:(q + 1) * T, :], dtype=np.float32)
        if q == 0:
            xhv = np.zeros((2, D), np.float32)
        else:
            xhv = np.ascontiguousarray(x[b, q * T - 2:q * T, :], dtype=np.float32)
        m = dict(common)
        m["xm"] = xm
        m["xh"] = xhv
        m["smask"] = (smq0 if q == 0 else sm0).reshape(H, 3 * T).copy()
        in_maps.append(m)
    return in_maps


def kernel(**inputs) -> np.ndarray:
    from concourse.bass_utils import run_bass_kernel_spmd

    if "nc" not in _CACHE:
        _CACHE["nc"] = _build_program()
    nc = _CACHE["nc"]
    in_maps = _host_inputs(**inputs)
    res = run_bass_kernel_spmd(nc, in_maps, list(range(NCORE)))
    outs = res.results
    full = np.zeros((2, 2048, D), np.float32)
    for core in range(NCORE):
        b, q = divmod(core, 4)
        full[b, q * T:(q + 1) * T, :] = outs[core]["out"]
    return full
